# revision 15
# baseline (speedup 1.0000x reference)
"""BAM-style attention block (avgpool8 -> 1024-token attention -> nearest-upsample + residual)
as a distributed Bass kernel on 8 TRN2 NeuronCores.

Sharding: core = b*2 + half  (b = batch 0..3, half = H-half 0..1).
Each core:
  phase 1: streams its x shard [512, 128, 256] per 128-channel group, avg-pools 8x8,
           and pipelines a pairwise AllGather of each pooled group with the streaming
  phase 2: q/k/v projections (bf16 matmuls), 512x1024 attention for its 512 query rows
  phase 3: re-streams x shard, adds the upsampled attention output, writes out shard
"""

import os
import numpy as np

B, C, H, W = 4, 512, 256, 256
DS = 8
HL = H // 2            # 128 rows per core
IL = HL // DS          # 16 pooled rows per core
WP = W // DS           # 32 pooled cols
NLOC = IL * WP         # 512 local tokens
N = 2 * NLOC           # 1024 tokens
K = C // 8             # 64
CG = C // 128          # 4 channel groups
NCHUNK = 8             # phase chunks per channel group (2 pooled rows each)
ROWS_PER_CHUNK = 16    # = 2 * DS

_CACHE = {}
TRACE = bool(int(os.environ.get("BAM_TRACE", "0")))
LAST_EXEC_NS = None


def _build():
    import concourse.bass as bass
    import concourse.tile as tile
    from concourse import bacc, mybir
    from concourse.masks import make_identity

    f32 = mybir.dt.float32
    bf16 = mybir.dt.bfloat16
    ADD = mybir.AluOpType.add
    AX = mybir.AxisListType.X
    AXY = mybir.AxisListType.XY
    Exp = mybir.ActivationFunctionType.Exp
    POOL_SCALE = 1.0 / (DS * DS)

    nc = bacc.Bacc("TRN2", target_bir_lowering=False, debug=False, num_devices=8)

    x_ext = nc.dram_tensor("x", [C, HL, W], f32, kind="ExternalInput")
    wq_ext = nc.dram_tensor("wq", [K, C], f32, kind="ExternalInput")
    bq_ext = nc.dram_tensor("bq", [1, K], f32, kind="ExternalInput")
    wk_ext = nc.dram_tensor("wk", [K, C], f32, kind="ExternalInput")
    bk_ext = nc.dram_tensor("bk", [1, K], f32, kind="ExternalInput")
    wv_ext = nc.dram_tensor("wv", [C, C], f32, kind="ExternalInput")
    bv_ext = nc.dram_tensor("bv", [1, C], f32, kind="ExternalInput")
    out_ext = nc.dram_tensor("out", [C, HL, W], f32, kind="ExternalOutput")

    with tile.TileContext(nc) as tc:
        with tc.tile_pool(name="persist", bufs=1) as persist, \
             tc.tile_pool(name="scratch", bufs=2) as scratch, \
             tc.tile_pool(name="p1", bufs=4) as p1, \
             tc.tile_pool(name="sm", bufs=8) as sm, \
             tc.tile_pool(name="p3", bufs=5) as p3, \
             tc.tile_pool(name="psum", bufs=3, space="PSUM") as psum, \
             tc.tile_pool(name="dram", bufs=1, space="DRAM") as dram:

            # ---- constants & weights (scalar-engine DMA ring; PE transposes) ----
            ident = persist.tile([128, 128], bf16, tag="ident")
            make_identity(nc, ident[:])
            ones = persist.tile([1, N], bf16, tag="ones")
            nc.vector.memset(ones[:], 1.0)

            def load_bias(ext, n):
                st = scratch.tile([1, n], f32, tag="bstage")
                nc.scalar.dma_start(out=st[:], in_=ext.ap())
                bb = persist.tile([1, n], bf16, tag=f"b_{ext.name}", name=f"b_{ext.name}")
                nc.scalar.copy(out=bb[:], in_=st[:])
                return bb

            bq_b = load_bias(bq_ext, K)
            bk_b = load_bias(bk_ext, K)
            bv_b = load_bias(bv_ext, C)

            def load_qk_weight(ext):
                st = scratch.tile([K, C], f32, tag="wstage")
                nc.scalar.dma_start(out=st[:], in_=ext.ap())
                wb = persist.tile([K, C], bf16, tag=f"wb_{ext.name}", name=f"wb_{ext.name}")
                nc.scalar.copy(out=wb[:], in_=st[:])
                wT = []
                for cg in range(CG):
                    ps = psum.tile([128, K], bf16, tag="mm")
                    nc.tensor.transpose(ps[:], wb[:, cg * 128:(cg + 1) * 128],
                                        ident[0:K, 0:K])
                    t = persist.tile([128, K], bf16, tag=f"wT_{ext.name}{cg}",
                                     name=f"wT_{ext.name}{cg}")
                    nc.scalar.copy(out=t[:], in_=ps[:])
                    wT.append(t)
                return wT

            wqT = load_qk_weight(wq_ext)
            wkT = load_qk_weight(wk_ext)

            # wvT[cg][c_loc, d] = Wv[d, cg*128 + c_loc]
            wvT = [persist.tile([128, C], bf16, tag=f"wvT{cg}", name=f"wvT{cg}")
                   for cg in range(CG)]
            for dt in range(CG):
                st = scratch.tile([128, C], f32, tag="wstage")
                nc.scalar.dma_start(out=st[:], in_=wv_ext.ap()[dt * 128:(dt + 1) * 128, :])
                wvb = scratch.tile([128, C], bf16, tag="wvstage")
                nc.scalar.copy(out=wvb[:], in_=st[:])
                for cg in range(CG):
                    ps = psum.tile([128, 128], bf16, tag="mm")
                    nc.tensor.transpose(ps[:], wvb[:, cg * 128:(cg + 1) * 128], ident[:])
                    nc.scalar.copy(out=wvT[cg][:, dt * 128:(dt + 1) * 128], in_=ps[:])

            # ---- phase 1: stream x + avg-pool; pipeline per-cg exchange + q/k matmuls ----
            # Tokens are kept in LOCAL-FIRST order throughout phase 2: columns
            # [0:512] are this core's tokens, [512:1024] the partner's. Softmax
            # and the final contraction are permutation-invariant over n, so the
            # global order never needs to be materialized.
            xf = [persist.tile([128, NLOC], f32, tag=f"xf{cg}", name=f"xf{cg}")
                  for cg in range(CG)]
            xfb_loc = [persist.tile([128, NLOC], bf16, tag=f"xfl{cg}", name=f"xfl{cg}")
                       for cg in range(CG)]
            # partner's half, recovered rank-agnostically as (h0 + h1) - local
            xfb_rem = [persist.tile([128, NLOC], bf16, tag=f"xfr{cg}", name=f"xfr{cg}")
                       for cg in range(CG)]
            xf_loc_d = dram.tile([CG, 128, NLOC], f32, tag="xf_loc")
            xf_all_d = dram.tile([CG, 2, 128, NLOC], f32, tag="xf_all")

            q_ps = psum.tile([K, NLOC], f32, tag="mm")
            k_ps = psum.tile([K, N], f32, tag="mm")

            for cg in range(CG):
                for ib in range(IL):
                    x1 = p1.tile([128, DS, W], f32, tag="x1")
                    nc.sync.dma_start(
                        out=x1[:],
                        in_=x_ext.ap()[cg * 128:(cg + 1) * 128,
                                       ib * DS:(ib + 1) * DS, :])
                    nc.vector.tensor_reduce(
                        out=xf[cg][:, ib * WP:(ib + 1) * WP],
                        in_=x1[:].rearrange("p h (j z) -> p j h z", z=DS),
                        axis=AXY, op=ADD)

                # local bf16 copy (applies the 1/64 pooling scale)
                nc.scalar.activation(out=xfb_loc[cg][:], in_=xf[cg][:],
                                     func=mybir.ActivationFunctionType.Copy,
                                     scale=POOL_SCALE)
                # local q/k partials (overlap the exchange)
                nc.tensor.matmul(q_ps[:], wqT[cg][:], xfb_loc[cg][:],
                                 start=(cg == 0), stop=False)
                nc.tensor.matmul(k_ps[:, :NLOC], wkT[cg][:], xfb_loc[cg][:],
                                 start=(cg == 0), stop=False)
                # pairwise exchange of this channel group's pooled block
                nc.gpsimd.dma_start(out=xf_loc_d[cg], in_=xf[cg][:])
                nc.gpsimd.collective_compute(
                    "AllGather",
                    mybir.AluOpType.bypass,
                    ins=[xf_loc_d[cg].opt()],
                    outs=[xf_all_d[cg].opt()],
                    replica_groups=[[0, 1], [2, 3], [4, 5], [6, 7]],
                )
                xfg = scratch.tile([128, N], f32, tag="xfg")
                for hf in range(2):
                    nc.scalar.dma_start(out=xfg[:, hf * NLOC:(hf + 1) * NLOC],
                                        in_=xf_all_d[cg, hf])
                hsum = scratch.tile([128, NLOC], f32, tag="hsum")
                nc.gpsimd.tensor_tensor(out=hsum[:], in0=xfg[:, :NLOC],
                                        in1=xfg[:, NLOC:], op=ADD)
                # xfb_rem = (h0 + h1) * (1/64) - xfb_loc
                nc.vector.scalar_tensor_tensor(
                    out=xfb_rem[cg][:], in0=hsum[:], scalar=POOL_SCALE,
                    in1=xfb_loc[cg][:],
                    op0=mybir.AluOpType.mult, op1=mybir.AluOpType.subtract)
                # remote k partial
                nc.tensor.matmul(k_ps[:, NLOC:], wkT[cg][:], xfb_rem[cg][:],
                                 start=(cg == 0), stop=False)

            # bias terms close the q/k accumulation groups
            nc.tensor.matmul(q_ps[:], bq_b[:], ones[:, :NLOC], start=False, stop=True)
            q_sb = persist.tile([K, NLOC], bf16, tag="q_sb")
            nc.scalar.copy(out=q_sb[:], in_=q_ps[:])
            for nh in range(2):
                sl = slice(nh * 512, (nh + 1) * 512)
                nc.tensor.matmul(k_ps[:, sl], bk_b[:], ones[:, :512],
                                 start=False, stop=True)
            k_sb = persist.tile([K, N], bf16, tag="k_sb")
            nc.scalar.copy(out=k_sb[:], in_=k_ps[:])

            # ---- vT projections: vT[nt][n_loc, d] = v[d, token nt*128+n_loc] + bv[d] ----
            # nt 0..3 = local tokens (ready before the last exchange lands),
            # nt 4..7 = partner tokens.
            vT = [persist.tile([128, C], bf16, tag=f"vT{nt}", name=f"vT{nt}")
                  for nt in range(N // 128)]
            for nt in range(N // 128):
                src = xfb_loc if nt < 4 else xfb_rem
                j = nt % 4
                v_ps = psum.tile([128, C], f32, tag="mm")
                for cg in range(CG):
                    nc.tensor.matmul(
                        v_ps[:],
                        src[cg][:, j * 128:(j + 1) * 128],
                        wvT[cg][:],
                        start=(cg == 0), stop=False)
                nc.tensor.matmul(v_ps[:], ones[:, :128], bv_b[:], start=False, stop=True)
                if nt % 2 == 0:
                    nc.vector.tensor_copy(out=vT[nt][:], in_=v_ps[:])
                else:
                    nc.scalar.copy(out=vT[nt][:], in_=v_ps[:])

            # ---- energy + softmax (rows = local m, cols = global n) ----
            # energies are tiny (|e/sqrt(K)| < ~0.05 for this model), so no max-subtraction
            attn = [persist.tile([128, N], bf16, tag=f"attn{mt}", name=f"attn{mt}")
                    for mt in range(4)]
            for mt in range(4):
                e_ps = psum.tile([128, N], f32, tag="mm")
                for nh in range(2):
                    sl = slice(nh * 512, (nh + 1) * 512)
                    nc.tensor.matmul(e_ps[:, sl],
                                     q_sb[:, mt * 128:(mt + 1) * 128],
                                     k_sb[:, sl], start=True, stop=True)
                rsum = sm.tile([128, 1], f32, tag="rsum")
                nc.scalar.activation(out=attn[mt][:], in_=e_ps[:], func=Exp,
                                     scale=K ** -0.5, accum_out=rsum[:])
                rinv = sm.tile([128, 1], f32, tag="rinv")
                nc.vector.reciprocal(rinv[:], rsum[:])
                nc.vector.tensor_scalar_mul(attn[mt][:], attn[mt][:], rinv[:])

            # ---- transpose attn -> attnT[nt][n_loc, m] ----
            attnT = [persist.tile([128, NLOC], bf16, tag=f"attnT{nt}", name=f"attnT{nt}")
                     for nt in range(N // 128)]
            for nt in range(N // 128):
                at_ps = psum.tile([128, NLOC], bf16, tag="mm")
                for mt in range(4):
                    nc.tensor.transpose(at_ps[:, mt * 128:(mt + 1) * 128],
                                        attn[mt][:, nt * 128:(nt + 1) * 128],
                                        ident[:])
                if nt % 2 == 0:
                    nc.vector.tensor_copy(out=attnT[nt][:], in_=at_ps[:])
                else:
                    nc.scalar.copy(out=attnT[nt][:], in_=at_ps[:])

            # ---- y[d, m] = sum_n v[d, n] attn[m, n] ----
            y = [persist.tile([128, NLOC], f32, tag=f"y{dt}", name=f"y{dt}")
                 for dt in range(CG)]
            for dt in range(CG):
                y_ps = psum.tile([128, NLOC], f32, tag="mm")
                for nt in range(N // 128):
                    nc.tensor.matmul(y_ps[:],
                                     vT[nt][:, dt * 128:(dt + 1) * 128],
                                     attnT[nt][:],
                                     start=(nt == 0), stop=(nt == N // 128 - 1))
                if dt % 2 == 0:
                    nc.vector.tensor_copy(out=y[dt][:], in_=y_ps[:])
                else:
                    nc.scalar.copy(out=y[dt][:], in_=y_ps[:])

            # ---- phase 3: out = x + upsample8(y) ----
            # loads on sync ring, adds on DVE, stores on scalar ring
            for cg in range(CG):
                for ib in range(NCHUNK):
                    x3 = p3.tile([128, ROWS_PER_CHUNK, W], f32, tag="x3")
                    nc.sync.dma_start(
                        out=x3[:],
                        in_=x_ext.ap()[cg * 128:(cg + 1) * 128,
                                       ib * ROWS_PER_CHUNK:(ib + 1) * ROWS_PER_CHUNK, :])
                    for i in range(2):
                        xv = x3[:, i * DS:(i + 1) * DS, :] \
                            .rearrange("p h (j z) -> p h j z", z=DS)
                        yv = y[cg][:, (ib * 2 + i) * WP:(ib * 2 + i + 1) * WP] \
                            [:, None, :, None].broadcast_to([128, DS, WP, DS])
                        nc.vector.tensor_tensor(out=xv, in0=xv, in1=yv, op=ADD)
                    nc.scalar.dma_start(
                        out=out_ext.ap()[cg * 128:(cg + 1) * 128,
                                         ib * ROWS_PER_CHUNK:(ib + 1) * ROWS_PER_CHUNK, :],
                        in_=x3[:])

    nc.finalize()
    return nc


def _get_nc():
    if "nc" not in _CACHE:
        _CACHE["nc"] = _build()
    return _CACHE["nc"]


def kernel(x, Wq, bq, Wk, bk, Wv, bv):
    global LAST_EXEC_NS
    from concourse.bass_utils import run_bass_kernel_spmd

    x = np.asarray(x, dtype=np.float32)
    Wq = np.asarray(Wq, dtype=np.float32)
    bq = np.asarray(bq, dtype=np.float32).reshape(1, K)
    Wk = np.asarray(Wk, dtype=np.float32)
    bk = np.asarray(bk, dtype=np.float32).reshape(1, K)
    Wv = np.asarray(Wv, dtype=np.float32)
    bv = np.asarray(bv, dtype=np.float32).reshape(1, C)

    nc = _get_nc()
    in_maps = []
    for core in range(8):
        b, half = core // 2, core % 2
        in_maps.append({
            "x": np.ascontiguousarray(x[b, :, half * HL:(half + 1) * HL, :]),
            "wq": Wq, "bq": bq, "wk": Wk, "bk": bk, "wv": Wv, "bv": bv,
        })

    res = run_bass_kernel_spmd(nc, in_maps, core_ids=list(range(8)), trace=TRACE)
    LAST_EXEC_NS = res.exec_time_ns

    out = np.empty((B, C, H, W), dtype=np.float32)
    for core in range(8):
        b, half = core // 2, core % 2
        out[b, :, half * HL:(half + 1) * HL, :] = res.results[core]["out"]
    return out


# revision 16
# speedup vs baseline: 1.0033x; 1.0033x over previous
"""BAM-style attention block (avgpool8 -> 1024-token attention -> nearest-upsample + residual)
as a distributed Bass kernel on 8 TRN2 NeuronCores.

Sharding: core = b*2 + half  (b = batch 0..3, half = H-half 0..1).
Each core:
  phase 1: streams its x shard [512, 128, 256] per 128-channel group, avg-pools 8x8,
           and pipelines a pairwise AllGather of each pooled group with the streaming
  phase 2: q/k/v projections (bf16 matmuls), 512x1024 attention for its 512 query rows
  phase 3: re-streams x shard, adds the upsampled attention output, writes out shard
"""

import os
import numpy as np

B, C, H, W = 4, 512, 256, 256
DS = 8
HL = H // 2            # 128 rows per core
IL = HL // DS          # 16 pooled rows per core
WP = W // DS           # 32 pooled cols
NLOC = IL * WP         # 512 local tokens
N = 2 * NLOC           # 1024 tokens
K = C // 8             # 64
CG = C // 128          # 4 channel groups
NCHUNK = 8             # phase chunks per channel group (2 pooled rows each)
ROWS_PER_CHUNK = 16    # = 2 * DS

_CACHE = {}
TRACE = bool(int(os.environ.get("BAM_TRACE", "0")))
LAST_EXEC_NS = None


def _build():
    import concourse.bass as bass
    import concourse.tile as tile
    from concourse import bacc, mybir
    from concourse.masks import make_identity

    f32 = mybir.dt.float32
    bf16 = mybir.dt.bfloat16
    ADD = mybir.AluOpType.add
    AX = mybir.AxisListType.X
    AXY = mybir.AxisListType.XY
    Exp = mybir.ActivationFunctionType.Exp
    POOL_SCALE = 1.0 / (DS * DS)

    nc = bacc.Bacc("TRN2", target_bir_lowering=False, debug=False, num_devices=8)

    x_ext = nc.dram_tensor("x", [C, HL, W], f32, kind="ExternalInput")
    wq_ext = nc.dram_tensor("wq", [K, C], f32, kind="ExternalInput")
    bq_ext = nc.dram_tensor("bq", [1, K], f32, kind="ExternalInput")
    wk_ext = nc.dram_tensor("wk", [K, C], f32, kind="ExternalInput")
    bk_ext = nc.dram_tensor("bk", [1, K], f32, kind="ExternalInput")
    wv_ext = nc.dram_tensor("wv", [C, C], f32, kind="ExternalInput")
    bv_ext = nc.dram_tensor("bv", [1, C], f32, kind="ExternalInput")
    out_ext = nc.dram_tensor("out", [C, HL, W], f32, kind="ExternalOutput")

    with tile.TileContext(nc) as tc:
        with tc.tile_pool(name="persist", bufs=1) as persist, \
             tc.tile_pool(name="scratch", bufs=2) as scratch, \
             tc.tile_pool(name="p1", bufs=4) as p1, \
             tc.tile_pool(name="sm", bufs=8) as sm, \
             tc.tile_pool(name="p3", bufs=5) as p3, \
             tc.tile_pool(name="psum", bufs=3, space="PSUM") as psum, \
             tc.tile_pool(name="dram", bufs=1, space="DRAM") as dram:

            # ---- constants & weights (scalar-engine DMA ring; PE transposes) ----
            ident = persist.tile([128, 128], bf16, tag="ident")
            make_identity(nc, ident[:])
            ones = persist.tile([1, N], bf16, tag="ones")
            nc.vector.memset(ones[:], 1.0)

            def load_bias(ext, n):
                st = scratch.tile([1, n], f32, tag="bstage")
                nc.scalar.dma_start(out=st[:], in_=ext.ap())
                bb = persist.tile([1, n], bf16, tag=f"b_{ext.name}", name=f"b_{ext.name}")
                nc.scalar.copy(out=bb[:], in_=st[:])
                return bb

            bq_b = load_bias(bq_ext, K)
            bk_b = load_bias(bk_ext, K)
            bv_b = load_bias(bv_ext, C)

            def load_qk_weight(ext):
                st = scratch.tile([K, C], f32, tag="wstage")
                nc.scalar.dma_start(out=st[:], in_=ext.ap())
                wb = persist.tile([K, C], bf16, tag=f"wb_{ext.name}", name=f"wb_{ext.name}")
                nc.scalar.copy(out=wb[:], in_=st[:])
                wT = []
                for cg in range(CG):
                    ps = psum.tile([128, K], bf16, tag="mm")
                    nc.tensor.transpose(ps[:], wb[:, cg * 128:(cg + 1) * 128],
                                        ident[0:K, 0:K])
                    t = persist.tile([128, K], bf16, tag=f"wT_{ext.name}{cg}",
                                     name=f"wT_{ext.name}{cg}")
                    nc.scalar.copy(out=t[:], in_=ps[:])
                    wT.append(t)
                return wT

            wqT = load_qk_weight(wq_ext)
            wkT = load_qk_weight(wk_ext)

            # wvT[cg][c_loc, d] = Wv[d, cg*128 + c_loc]
            wvT = [persist.tile([128, C], bf16, tag=f"wvT{cg}", name=f"wvT{cg}")
                   for cg in range(CG)]
            for dt in range(CG):
                st = scratch.tile([128, C], f32, tag="wstage")
                nc.scalar.dma_start(out=st[:], in_=wv_ext.ap()[dt * 128:(dt + 1) * 128, :])
                wvb = scratch.tile([128, C], bf16, tag="wvstage")
                nc.scalar.copy(out=wvb[:], in_=st[:])
                for cg in range(CG):
                    ps = psum.tile([128, 128], bf16, tag="mm")
                    nc.tensor.transpose(ps[:], wvb[:, cg * 128:(cg + 1) * 128], ident[:])
                    nc.scalar.copy(out=wvT[cg][:, dt * 128:(dt + 1) * 128], in_=ps[:])

            # ---- phase 1: stream x + avg-pool; pipeline per-cg exchange + q/k matmuls ----
            # Tokens are kept in LOCAL-FIRST order throughout phase 2: columns
            # [0:512] are this core's tokens, [512:1024] the partner's. Softmax
            # and the final contraction are permutation-invariant over n, so the
            # global order never needs to be materialized.
            xf = [persist.tile([128, NLOC], f32, tag=f"xf{cg}", name=f"xf{cg}")
                  for cg in range(CG)]
            xfb_loc = [persist.tile([128, NLOC], bf16, tag=f"xfl{cg}", name=f"xfl{cg}")
                       for cg in range(CG)]
            # partner's half, recovered rank-agnostically as (h0 + h1) - local
            xfb_rem = [persist.tile([128, NLOC], bf16, tag=f"xfr{cg}", name=f"xfr{cg}")
                       for cg in range(CG)]
            xf_loc_d = dram.tile([CG, 128, NLOC], f32, tag="xf_loc")
            xf_all_d = dram.tile([CG, 2, 128, NLOC], f32, tag="xf_all")

            q_ps = psum.tile([K, NLOC], f32, tag="mm")
            k_ps = psum.tile([K, N], f32, tag="mm")

            for cg in range(CG):
                for ib in range(IL):
                    x1 = p1.tile([128, DS, W], f32, tag="x1")
                    nc.sync.dma_start(
                        out=x1[:],
                        in_=x_ext.ap()[cg * 128:(cg + 1) * 128,
                                       ib * DS:(ib + 1) * DS, :])
                    nc.vector.tensor_reduce(
                        out=xf[cg][:, ib * WP:(ib + 1) * WP],
                        in_=x1[:].rearrange("p h (j z) -> p j h z", z=DS),
                        axis=AXY, op=ADD)

                # local bf16 copy (applies the 1/64 pooling scale)
                nc.scalar.activation(out=xfb_loc[cg][:], in_=xf[cg][:],
                                     func=mybir.ActivationFunctionType.Copy,
                                     scale=POOL_SCALE)
                # local q/k partials (overlap the exchange)
                nc.tensor.matmul(q_ps[:], wqT[cg][:], xfb_loc[cg][:],
                                 start=(cg == 0), stop=False)
                nc.tensor.matmul(k_ps[:, :NLOC], wkT[cg][:], xfb_loc[cg][:],
                                 start=(cg == 0), stop=False)
                # pairwise exchange of this channel group's pooled block
                nc.gpsimd.dma_start(out=xf_loc_d[cg], in_=xf[cg][:])
                nc.gpsimd.collective_compute(
                    "AllGather",
                    mybir.AluOpType.bypass,
                    ins=[xf_loc_d[cg].opt()],
                    outs=[xf_all_d[cg].opt()],
                    replica_groups=[[0, 1], [2, 3], [4, 5], [6, 7]],
                )
                xfg = scratch.tile([128, N], f32, tag="xfg")
                for hf in range(2):
                    nc.scalar.dma_start(out=xfg[:, hf * NLOC:(hf + 1) * NLOC],
                                        in_=xf_all_d[cg, hf])
                hsum = scratch.tile([128, NLOC], f32, tag="hsum")
                nc.gpsimd.tensor_tensor(out=hsum[:], in0=xfg[:, :NLOC],
                                        in1=xfg[:, NLOC:], op=ADD)
                # remote half = (h0 + h1) - local, then scale+cast on ACT so the
                # DVE stream stays pure pooling (an in-order DVE wait here would
                # stall phase-1 slot recycling)
                rem_raw = scratch.tile([128, NLOC], f32, tag="rem_raw")
                nc.gpsimd.tensor_tensor(out=rem_raw[:], in0=hsum[:],
                                        in1=xf[cg][:], op=mybir.AluOpType.subtract)
                nc.scalar.activation(out=xfb_rem[cg][:], in_=rem_raw[:],
                                     func=mybir.ActivationFunctionType.Copy,
                                     scale=POOL_SCALE)
                # remote k partial
                nc.tensor.matmul(k_ps[:, NLOC:], wkT[cg][:], xfb_rem[cg][:],
                                 start=(cg == 0), stop=False)

            # bias terms close the q/k accumulation groups
            nc.tensor.matmul(q_ps[:], bq_b[:], ones[:, :NLOC], start=False, stop=True)
            q_sb = persist.tile([K, NLOC], bf16, tag="q_sb")
            nc.scalar.copy(out=q_sb[:], in_=q_ps[:])
            for nh in range(2):
                sl = slice(nh * 512, (nh + 1) * 512)
                nc.tensor.matmul(k_ps[:, sl], bk_b[:], ones[:, :512],
                                 start=False, stop=True)
            k_sb = persist.tile([K, N], bf16, tag="k_sb")
            nc.scalar.copy(out=k_sb[:], in_=k_ps[:])

            # ---- vT projections: vT[nt][n_loc, d] = v[d, token nt*128+n_loc] + bv[d] ----
            # nt 0..3 = local tokens (ready before the last exchange lands),
            # nt 4..7 = partner tokens.
            vT = [persist.tile([128, C], bf16, tag=f"vT{nt}", name=f"vT{nt}")
                  for nt in range(N // 128)]
            for nt in range(N // 128):
                src = xfb_loc if nt < 4 else xfb_rem
                j = nt % 4
                v_ps = psum.tile([128, C], f32, tag="mm")
                for cg in range(CG):
                    nc.tensor.matmul(
                        v_ps[:],
                        src[cg][:, j * 128:(j + 1) * 128],
                        wvT[cg][:],
                        start=(cg == 0), stop=False)
                nc.tensor.matmul(v_ps[:], ones[:, :128], bv_b[:], start=False, stop=True)
                if nt % 2 == 0:
                    nc.vector.tensor_copy(out=vT[nt][:], in_=v_ps[:])
                else:
                    nc.scalar.copy(out=vT[nt][:], in_=v_ps[:])

            # ---- energy + softmax (rows = local m, cols = global n) ----
            # energies are tiny (|e/sqrt(K)| < ~0.05 for this model), so no max-subtraction
            attn = [persist.tile([128, N], bf16, tag=f"attn{mt}", name=f"attn{mt}")
                    for mt in range(4)]
            for mt in range(4):
                e_ps = psum.tile([128, N], f32, tag="mm")
                for nh in range(2):
                    sl = slice(nh * 512, (nh + 1) * 512)
                    nc.tensor.matmul(e_ps[:, sl],
                                     q_sb[:, mt * 128:(mt + 1) * 128],
                                     k_sb[:, sl], start=True, stop=True)
                rsum = sm.tile([128, 1], f32, tag="rsum")
                nc.scalar.activation(out=attn[mt][:], in_=e_ps[:], func=Exp,
                                     scale=K ** -0.5, accum_out=rsum[:])
                rinv = sm.tile([128, 1], f32, tag="rinv")
                nc.vector.reciprocal(rinv[:], rsum[:])
                nc.vector.tensor_scalar_mul(attn[mt][:], attn[mt][:], rinv[:])

            # ---- transpose attn -> attnT[nt][n_loc, m] ----
            attnT = [persist.tile([128, NLOC], bf16, tag=f"attnT{nt}", name=f"attnT{nt}")
                     for nt in range(N // 128)]
            for nt in range(N // 128):
                at_ps = psum.tile([128, NLOC], bf16, tag="mm")
                for mt in range(4):
                    nc.tensor.transpose(at_ps[:, mt * 128:(mt + 1) * 128],
                                        attn[mt][:, nt * 128:(nt + 1) * 128],
                                        ident[:])
                if nt % 2 == 0:
                    nc.vector.tensor_copy(out=attnT[nt][:], in_=at_ps[:])
                else:
                    nc.scalar.copy(out=attnT[nt][:], in_=at_ps[:])

            # ---- y[d, m] = sum_n v[d, n] attn[m, n] ----
            y = [persist.tile([128, NLOC], f32, tag=f"y{dt}", name=f"y{dt}")
                 for dt in range(CG)]
            for dt in range(CG):
                y_ps = psum.tile([128, NLOC], f32, tag="mm")
                for nt in range(N // 128):
                    nc.tensor.matmul(y_ps[:],
                                     vT[nt][:, dt * 128:(dt + 1) * 128],
                                     attnT[nt][:],
                                     start=(nt == 0), stop=(nt == N // 128 - 1))
                if dt % 2 == 0:
                    nc.vector.tensor_copy(out=y[dt][:], in_=y_ps[:])
                else:
                    nc.scalar.copy(out=y[dt][:], in_=y_ps[:])

            # ---- phase 3: out = x + upsample8(y) ----
            # loads on sync ring, adds on DVE, stores on scalar ring
            for cg in range(CG):
                for ib in range(NCHUNK):
                    x3 = p3.tile([128, ROWS_PER_CHUNK, W], f32, tag="x3")
                    nc.sync.dma_start(
                        out=x3[:],
                        in_=x_ext.ap()[cg * 128:(cg + 1) * 128,
                                       ib * ROWS_PER_CHUNK:(ib + 1) * ROWS_PER_CHUNK, :])
                    for i in range(2):
                        xv = x3[:, i * DS:(i + 1) * DS, :] \
                            .rearrange("p h (j z) -> p h j z", z=DS)
                        yv = y[cg][:, (ib * 2 + i) * WP:(ib * 2 + i + 1) * WP] \
                            [:, None, :, None].broadcast_to([128, DS, WP, DS])
                        nc.vector.tensor_tensor(out=xv, in0=xv, in1=yv, op=ADD)
                    nc.scalar.dma_start(
                        out=out_ext.ap()[cg * 128:(cg + 1) * 128,
                                         ib * ROWS_PER_CHUNK:(ib + 1) * ROWS_PER_CHUNK, :],
                        in_=x3[:])

    nc.finalize()
    return nc


def _get_nc():
    if "nc" not in _CACHE:
        _CACHE["nc"] = _build()
    return _CACHE["nc"]


def kernel(x, Wq, bq, Wk, bk, Wv, bv):
    global LAST_EXEC_NS
    from concourse.bass_utils import run_bass_kernel_spmd

    x = np.asarray(x, dtype=np.float32)
    Wq = np.asarray(Wq, dtype=np.float32)
    bq = np.asarray(bq, dtype=np.float32).reshape(1, K)
    Wk = np.asarray(Wk, dtype=np.float32)
    bk = np.asarray(bk, dtype=np.float32).reshape(1, K)
    Wv = np.asarray(Wv, dtype=np.float32)
    bv = np.asarray(bv, dtype=np.float32).reshape(1, C)

    nc = _get_nc()
    in_maps = []
    for core in range(8):
        b, half = core // 2, core % 2
        in_maps.append({
            "x": np.ascontiguousarray(x[b, :, half * HL:(half + 1) * HL, :]),
            "wq": Wq, "bq": bq, "wk": Wk, "bk": bk, "wv": Wv, "bv": bv,
        })

    res = run_bass_kernel_spmd(nc, in_maps, core_ids=list(range(8)), trace=TRACE)
    LAST_EXEC_NS = res.exec_time_ns

    out = np.empty((B, C, H, W), dtype=np.float32)
    for core in range(8):
        b, half = core // 2, core % 2
        out[b, :, half * HL:(half + 1) * HL, :] = res.results[core]["out"]
    return out
